# revision 1
# baseline (speedup 1.0000x reference)
"""AttentionPairBias kernel for 8 Trainium2 NeuronCores.

Sharding strategy: data-parallel over the query/sequence dimension i.
Each core c owns rows i in [c*128, (c+1)*128) of the n=1024 sequence.
 - pairwise_repr is sequence-sharded along i (67 MB/core instead of 537 MB
   replicated) -> each core computes its [h, 128, n] slice of the LN+Wb bias.
 - single_repr / weights are replicated (tiny); k/v are recomputed per core.
 - Each core produces final output rows [c*128:(c+1)*128, :] independently,
   so NO collective is needed; the host concatenates the 8 slices.

Shapes (hardcoded): b=1, n=1024, ds=384, dp=128, h=16, dh=64, inner=1024.
"""

import numpy as np
import jax
import jax.numpy as jnp

EPS = 1e-5
N = 1024
DS = 384
DP = 128
H = 16
DH = 64
INNER = H * DH
NCORES = 8
ROWS = N // NCORES  # 128 query rows per core


def _shard_fn(pair, ab_slice, sr_full, sr_slice, ln_gamma, ln_beta,
              Wb, Wq, bq, Wk, Wv, Wg, Wo):
    """Compute output rows for one i-slice. All args are device arrays.

    pair:     [ROWS, N, DP]  pairwise_repr rows for this slice
    ab_slice: [ROWS, N]      attn_bias rows for this slice
    sr_full:  [N, DS]        full single_repr (for k, v)
    sr_slice: [ROWS, DS]     single_repr rows for this slice (q, gates)
    """
    # --- pairwise -> attention bias (LayerNorm + LinearNoBias)
    mu = jnp.mean(pair, axis=-1, keepdims=True)
    var = jnp.mean(jnp.square(pair - mu), axis=-1, keepdims=True)
    pw = (pair - mu) * jax.lax.rsqrt(var + EPS) * ln_gamma + ln_beta
    bias = jnp.einsum('ijd,dh->hij', pw, Wb)          # [H, ROWS, N]
    bias = bias + ab_slice[None, :, :]

    # --- attention
    scale = DH ** -0.5
    q = (sr_slice @ Wq + bq).reshape(ROWS, H, DH).transpose(1, 0, 2)
    k = (sr_full @ Wk).reshape(N, H, DH).transpose(1, 0, 2)
    v = (sr_full @ Wv).reshape(N, H, DH).transpose(1, 0, 2)

    scores = jnp.einsum('hid,hjd->hij', q, k) * scale + bias
    m = jnp.max(scores, axis=-1, keepdims=True)
    e = jnp.exp(scores - m)
    attn = e / jnp.sum(e, axis=-1, keepdims=True)
    out = jnp.einsum('hij,hjd->hid', attn, v)          # [H, ROWS, DH]
    out = out.transpose(1, 0, 2).reshape(ROWS, INNER)

    gates = jax.nn.sigmoid(sr_slice @ Wg)
    out = out * gates
    return out @ Wo                                    # [ROWS, DS]


_JIT = jax.jit(_shard_fn)


def kernel(single_repr, pairwise_repr, attn_bias, ln_gamma, ln_beta,
           Wb, Wq, bq, Wk, Wv, Wg, Wo):
    single_repr = np.asarray(single_repr)
    pairwise_repr = np.asarray(pairwise_repr)
    attn_bias = np.asarray(attn_bias)

    devs = jax.devices()[:NCORES]
    sr = single_repr[0]        # [N, DS]
    pw = pairwise_repr[0]      # [N, N, DP]
    ab = attn_bias[0]          # [N, N]

    weights = (np.asarray(ln_gamma), np.asarray(ln_beta), np.asarray(Wb),
               np.asarray(Wq), np.asarray(bq), np.asarray(Wk),
               np.asarray(Wv), np.asarray(Wg), np.asarray(Wo))

    futures = []
    for c, dev in enumerate(devs):
        lo, hi = c * ROWS, (c + 1) * ROWS
        args = (pw[lo:hi], ab[lo:hi], sr, sr[lo:hi]) + weights
        dargs = [jax.device_put(a, dev) for a in args]
        futures.append(_JIT(*dargs))

    parts = [np.asarray(f) for f in futures]
    out = np.concatenate(parts, axis=0)[None]          # [1, N, DS]
    return out.astype(np.float32)
